# revision 23
# baseline (speedup 1.0000x reference)
"""Trainium2 Bass kernel for nn_CausalSelfAttention (tensor-parallel over heads, 8 cores).

Contract: kernel(**inputs) takes FULL unsharded numpy inputs and returns the
FULL output [1, 2048, 1024] float32. Internally: shards over 8 NeuronCores
(2 heads each, Wq/Wk/Wv column-sharded, Wo row-sharded), runs one SPMD Bass
program via run_bass_kernel_spmd, and sums the 8 partial Wo products on the
host (the row-parallel unshard).

Compute structure per core (heads 2c, 2c+1), chunk-pipelined so that
attention for query-chunk c overlaps the q/k projection + norm of chunk c+1
(keeps PE continuously busy and lets ACT's softmax-exp hide under PE work):
  - host passes x pre-transposed (xT [D, T]) and rotary tables with the
    s_eff scale folded in (rota/rotb [128, T], bf16)
  - v projected first (contraction-outer, rides the x DMA), then per chunk:
    q/k projected free-dim-outer ([128, 512] PSUM accumulators), cosine-norm
    via squared-sums matmul + exp(-0.5 ln) on ACT, rotary as windowed DVE
    muls (the partition-block swap is expressed as partition-offset operand
    windows, no data movement)
  - attention in S^T layout: S^T[ts, tq] = k-hat.T @ q-hat (two heads packed
    via tile_position), P^T = exp(0.12 S^T) on ACT (bf16 out), causal
    diagonal blocks masked by affine_select on Pool, AV as V^T P^T with
    64 ones-rows per head riding the same rhs stream to accumulate the
    softmax denominator Z (broadcast 64x) for free
  - tail: 1/Z via reciprocal_approx_fast straight from PSUM, normalize-mul
    reads y from PSUM directly, Wo per 128-row block, PSUM->SBUF move on
    DVE, DMA out
Matmul operands are bf16 (TensorE runs fp32 at quarter rate); accumulation is
fp32 in PSUM; softmax stats kept in fp32.
"""

import os
import sys
import types

import numpy as np
import ml_dtypes

for _p in ("/opt/trn_rl_repo", "/root/.axon_site/_ro/trn_rl_repo"):
    if os.path.isdir(_p) and _p not in sys.path:
        sys.path.append(_p)

import concourse.bass as bass
import concourse.mybir as mybir
import concourse.tile as tile
from concourse.bass_utils import run_bass_kernel_spmd

F32 = mybir.dt.float32
BF16 = mybir.dt.bfloat16
FP8 = mybir.dt.float8e4
NPBF16 = ml_dtypes.bfloat16
FP8_AV = False  # fp8 P/V passes quantization error straight through to the
                # output (attention output is a weighted mean of random-sign
                # values: signal and noise shrink together) — measured 3.1e-2
                # rel err vs the 2e-2 gate. Keep everything bf16.
NCORES = 8
T = 2048
D = 1024
NH = 16
HD = 64
HPC = NH // NCORES   # heads per core
EPC = HPC * HD       # projection cols per core
ATTN_SCALE = 0.12
NT = T // 512
NK = D // 128

LAST = {}


def _register_ntff_hook():
    """Best-effort: register the axon NTFF profile hook if the image's antenv
    lacks axon_hooks (profiling only; compile/run work without it)."""
    try:
        import antenv.axon_hooks  # noqa: F401
        return
    except ImportError:
        pass
    try:
        import trn_agent_boot.trn_boot as tb

        mod = types.ModuleType("antenv.axon_hooks")
        holder = {}
        mod.set_axon_ntff_profile_hook = lambda h: holder.__setitem__("h", h)
        mod.get_axon_ntff_profile_hook = lambda: holder.get("h")
        sys.modules["antenv.axon_hooks"] = mod
        mod.set_axon_ntff_profile_hook(
            tb._ntff_profile_via_ctypes("/opt/axon/libaxon_pjrt.so")
        )
    except Exception:
        pass


def _split_ctrl_waits(nc, k_default=1):
    """The container's walrus build rejects instructions carrying more than one
    semaphore sync-wait; hoist extra waits onto single-wait NoOps that precede
    the instruction on the same engine queue (AND semantics preserved)."""
    n_nops = 0
    for f in nc.m.functions:
        for blk in f.blocks:
            new, changed = [], False
            for inst in list(blk.instructions):
                si = inst.sync_info
                waits = list(si.on_wait) if si is not None else []
                kmax = 1 if isinstance(inst, mybir.InstDrain) else k_default
                if len(waits) > kmax:
                    for k, w in enumerate(waits[:-kmax]):
                        nop = mybir.InstNoOp(name=f"{inst.name}-sw{k}", ins=[], outs=[])
                        nop.engine = inst.engine
                        nop.sync_info = mybir.SyncInfo(on_wait=[w], on_update=[])
                        new.append(nop)
                        n_nops += 1
                    inst.sync_info = mybir.SyncInfo(
                        on_wait=list(waits[-kmax:]), on_update=list(si.on_update)
                    )
                    changed = True
                new.append(inst)
            if changed:
                blk.instructions = new
    return n_nops


def _build_nc():
    nc = bass.Bass("TRN2", target_bir_lowering=False, debug=False, num_devices=NCORES)

    xT_d = nc.dram_tensor("xT", [D, T], BF16, kind="ExternalInput")
    wq_d = nc.dram_tensor("wq", [D, EPC], BF16, kind="ExternalInput")
    wk_d = nc.dram_tensor("wk", [D, EPC], BF16, kind="ExternalInput")
    wv_d = nc.dram_tensor("wv", [D, EPC], BF16, kind="ExternalInput")
    wo_d = nc.dram_tensor("wo", [EPC, D], BF16, kind="ExternalInput")
    rota_d = nc.dram_tensor("rota", [EPC, T], BF16, kind="ExternalInput")
    # rotb doubled per chunk ([p, chunk, {q,k}, t]) so the combined q|k
    # [128, 1024] windowed muls read one contiguous slice
    rotb_d = nc.dram_tensor("rotb", [EPC, 2 * T], BF16, kind="ExternalInput")
    hselw_d = nc.dram_tensor("hselw", [128, 128], BF16, kind="ExternalInput")
    out_d = nc.dram_tensor("out", [T, D], BF16, kind="ExternalOutput")

    with tile.TileContext(nc) as tc:
        with (
            tc.tile_pool(name="wt", bufs=1) as wt,
            tc.tile_pool(name="sb", bufs=2) as sbp,      # norm-chain intermediates
            tc.tile_pool(name="at", bufs=4) as atp,      # exp outputs
            tc.tile_pool(name="tl", bufs=2) as tlp,      # tail tiles
            tc.tile_pool(name="ob", bufs=4) as obp,      # out staging
            tc.tile_pool(name="ps", bufs=3, space="PSUM") as psp,   # 3x[128,1024]
            tc.tile_pool(name="py", bufs=1, space="PSUM") as pyp,   # 2x[128,512]
        ):
            # ---- constants / weights ----
            wq_s = wt.tile([128, D], BF16, tag="wq")
            wk_s = wt.tile([128, D], BF16, tag="wk")
            wv_s = wt.tile([128, D], BF16, tag="wv")
            wo_s = wt.tile([EPC, D], BF16, tag="wo")
            rota = wt.tile([EPC, T], BF16, tag="rota")
            rotb = wt.tile([EPC, 2 * T], BF16, tag="rotb")
            hselw = wt.tile([128, 128], BF16, tag="hselw")
            eps = wt.tile([128, 1], F32, tag="eps")

            def load_w(w_s, w_d):
                nc.sync.dma_start(
                    w_s[:].rearrange("p (i f) -> p i f", i=NK),
                    w_d[:].rearrange("(i p) f -> p i f", p=128),
                )

            load_w(wv_s, wv_d)
            xc = []
            for i in range(NK):
                t_ = wt.tile([128, T], BF16, tag=f"xc{i}", name=f"xc_{i}")
                nc.sync.dma_start(t_[:], xT_d[128 * i : 128 * (i + 1), :])
                xc.append(t_)
            load_w(wq_s, wq_d)
            load_w(wk_s, wk_d)
            nc.sync.dma_start(wo_s[:], wo_d[:])
            nc.sync.dma_start(rota[:], rota_d[:])
            nc.sync.dma_start(rotb[:], rotb_d[:])
            nc.sync.dma_start(hselw[:], hselw_d[:])
            nc.gpsimd.memset(eps[:], 1e-12)

            qrot = wt.tile([EPC, T], BF16, tag="qrot")
            krot = wt.tile([EPC, T], BF16, tag="krot")
            vT_raw = wt.tile([EPC, T], BF16, tag="vraw")

            # ---- v projection: contraction-outer, rides the x DMA ----
            vps = [
                psp.tile([128, 1024], F32, tag="ps", name=f"vps_{p}")
                for p in range(2)
            ]
            for i in range(NK):
                for n in range(NT):
                    nc.tensor.matmul(
                        vps[n // 2][:, 512 * (n % 2) : 512 * (n % 2 + 1)],
                        wv_s[:, 128 * i : 128 * (i + 1)],
                        xc[i][:, 512 * n : 512 * (n + 1)],
                        start=(i == 0),
                        stop=(i == NK - 1),
                    )
            for p in range(2):
                nc.scalar.copy(vT_raw[:, 1024 * p : 1024 * (p + 1)], vps[p][:])

            # ---- vext: v in natural layout + 64 ones-rows per head ----
            # per 128-block j the lhsT is [128 ts, (h, v64|ones64)]; the ones
            # rows ride the AV rhs stream to accumulate Z broadcast 64x.
            # fp8 layout pairs two ts-blocks per DoubleRow matmul:
            # (jp, ktile, h, col), lhsT free = (ktile, 128)
            av_dt = FP8 if FP8_AV else BF16
            vext = wt.tile([128, (T // 128) * 256], av_dt, tag="vext")
            nc.gpsimd.memset(vext[:], 1.0)
            ident = wt.tile([128, 128], BF16, tag="ident")
            nc.gpsimd.memset(ident[:], 0.0)
            nc.gpsimd.affine_select(
                out=ident[:],
                in_=ident[:],
                compare_op=mybir.AluOpType.not_equal,
                fill=1.0,
                base=0,
                pattern=[[-1, 128]],
                channel_multiplier=1,
            )
            if FP8_AV:
                vv = vext[:].rearrange(
                    "p (jp i h f) -> p jp i h f", i=2, h=HPC, f=128
                )
            else:
                vv = vext[:].rearrange("p (j h f) -> p j h f", h=HPC, f=128)

            def emit_transposes():
                for j in range(T // 128):
                    tp_ = psp.tile([128, 128], BF16, tag="ps", name=f"vtp_{j}")
                    nc.tensor.transpose(
                        tp_[:], vT_raw[:, 128 * j : 128 * (j + 1)], ident[:]
                    )
                    dst = (
                        vv[:, j // 2, j % 2, :, 0:64]
                        if FP8_AV
                        else vv[:, j, :, 0:64]
                    )
                    nc.scalar.copy(
                        dst, tp_[:].rearrange("p (h f) -> p h f", h=HPC)
                    )

            # ---- per-chunk q|k projection + cosine-norm + rotary ----
            # q and k share one [128, 1024] stream (q cols 0:512, k 512:1024)
            # to halve the per-chunk instruction count on DVE/ACT
            def proj_norm_rot(c):
                cq = slice(512 * c, 512 * (c + 1))
                c2 = slice(1024 * c, 1024 * (c + 1))
                pj = psp.tile([128, 1024], F32, tag="ps", name=f"pj_{c}")
                for half, w_s in ((0, wq_s), (1, wk_s)):
                    for i in range(NK):
                        nc.tensor.matmul(
                            pj[:, 512 * half : 512 * (half + 1)],
                            w_s[:, 128 * i : 128 * (i + 1)],
                            xc[i][:, cq],
                            start=(i == 0),
                            stop=(i == NK - 1),
                        )
                raw = sbp.tile([128, 1024], BF16, tag="raw", bufs=3, name=f"raw_{c}")
                nc.vector.tensor_copy(raw[:], pj[:])
                sq = sbp.tile([128, 1024], BF16, tag="sq", name=f"sq_{c}")
                nc.vector.tensor_mul(sq[:], raw[:], raw[:])
                ssb = psp.tile([128, 1024], F32, tag="ps", name=f"ssb_{c}")
                for half in range(2):
                    nc.tensor.matmul(
                        ssb[:, 512 * half : 512 * (half + 1)],
                        hselw[:],
                        sq[:, 512 * half : 512 * (half + 1)],
                        start=True,
                        stop=True,
                    )
                lw = sbp.tile([128, 1024], F32, tag="lw", name=f"lw_{c}")
                nc.scalar.activation(
                    lw[:], ssb[:], mybir.ActivationFunctionType.Ln, bias=eps[:]
                )
                rw = sbp.tile([128, 1024], BF16, tag="rw", name=f"rw_{c}")
                nc.scalar.activation(
                    rw[:], lw[:], mybir.ActivationFunctionType.Exp, scale=-0.5
                )
                qn = sbp.tile([128, 1024], BF16, tag="qn", name=f"qn_{c}")
                nc.vector.tensor_mul(qn[:], raw[:], rw[:])
                # rotary: rot = qn*rota + swap(qn)*rotb. The partition swap
                # rides the OUTPUT AP of four windowed muls (walrus only
                # requires the two SBUF *inputs* of a TensorTensor to share
                # a start partition; the output may shift), with rotb
                # pre-swapped on the host so inputs stay aligned.
                swp = sbp.tile([128, 1024], BF16, tag="swp", name=f"swp_{c}")
                for (a, b) in ((0, 32), (32, 0), (64, 96), (96, 64)):
                    nc.vector.tensor_mul(
                        swp[a : a + 32, :], qn[b : b + 32, :], rotb[b : b + 32, c2]
                    )
                for half, rot in ((0, qrot), (1, krot)):
                    hf = slice(512 * half, 512 * (half + 1))
                    nc.vector.tensor_mul(rot[:, cq], qn[:, hf], rota[:, cq])
                    nc.vector.tensor_add(rot[:, cq], rot[:, cq], swp[:, hf])

            # ---- attention core (S^T layout) for chunk c ----
            # fp8 path: exp writes e4m3 P into pair tiles [ktile 2, h, tq];
            # AV runs DoubleRow (contraction 256 = two ts-blocks per call)
            def attention_core(c):
                nts = 4 * c + 4
                cq = slice(512 * c, 512 * (c + 1))
                pyh = [
                    pyp.tile([128, 512], F32, name=f"py{h}_{c}", tag=f"py{h}")
                    for h in range(HPC)
                ]
                pt_pair = None
                for j in range(nts):
                    m = j - 4 * c
                    ps2 = psp.tile([128, 1024], F32, tag="ps", name=f"s_{c}_{j}")
                    for h in range(HPC):
                        hs = slice(64 * h, 64 * (h + 1))
                        nc.tensor.matmul(
                            ps2[:, 512 * h : 512 * (h + 1)],
                            krot[hs, 128 * j : 128 * (j + 1)],
                            qrot[hs, cq],
                            start=True,
                            stop=True,
                            tile_position=(64 * h, 0),
                        )
                    if FP8_AV:
                        if j % 2 == 0:
                            pt_pair = atp.tile(
                                [128, 2048], FP8, tag="pt", name=f"pt_{c}_{j // 2}"
                            )
                        pb = 1024 * (j % 2)
                    else:
                        pt_pair = atp.tile(
                            [128, 1024], BF16, tag="pt", name=f"pt_{c}_{j}"
                        )
                        pb = 0
                    psl = pt_pair[:, pb : pb + 1024]
                    if m >= 1:
                        # diagonal block: columns tq < 128m are fully masked
                        # per head — skip their exp; affine_select's fill
                        # zeroes the unwritten (stale) region anyway
                        for h in range(HPC):
                            sl = slice(
                                pb + 512 * h + 128 * m, pb + 512 * (h + 1)
                            )
                            s0 = slice(512 * h + 128 * m, 512 * (h + 1))
                            nc.scalar.activation(
                                pt_pair[:, sl], ps2[:, s0],
                                mybir.ActivationFunctionType.Exp,
                                scale=ATTN_SCALE,
                            )
                    else:
                        nc.scalar.activation(
                            psl, ps2[:], mybir.ActivationFunctionType.Exp,
                            scale=ATTN_SCALE,
                        )
                    if m >= 0:
                        # causal: keep pt[x, (h, y)] only where y >= x + 128*m
                        nc.gpsimd.affine_select(
                            out=psl,
                            in_=psl,
                            compare_op=mybir.AluOpType.is_ge,
                            fill=0.0,
                            base=-128 * m,
                            pattern=[[0, 2], [1, 512]],
                            channel_multiplier=-1,
                        )
                    if FP8_AV:
                        if j % 2 == 1:
                            jp = j // 2
                            ptv = pt_pair[:].rearrange(
                                "p (i h t) -> p i h t", i=2, h=HPC
                            )
                            for h in range(HPC):
                                nc.tensor.matmul(
                                    pyh[h][:],
                                    vv[:, jp, :, h, :],
                                    ptv[:, :, h, :],
                                    start=(jp == 0),
                                    stop=(jp == nts // 2 - 1),
                                    perf_mode=mybir.MatmulPerfMode.DoubleRow,
                                )
                    else:
                        for h in range(HPC):
                            nc.tensor.matmul(
                                pyh[h][:],
                                vext[:, 256 * j + 128 * h : 256 * j + 128 * (h + 1)],
                                pt_pair[:, 512 * h : 512 * (h + 1)],
                                start=(j == 0),
                                stop=(j == nts - 1),
                            )
                return pyh

            # ---- tail: 1/Z = exp(-ln Z) on ACT (same table set as the
            # softmax exp; custom-DVE/divide ISA is unavailable in this
            # toolchain), normalize-mul straight from PSUM on DVE ----
            def tail_norm(c, pyh):
                yt = tlp.tile([128, 512], BF16, tag="yt", name=f"yt_{c}")
                for h in range(HPC):
                    hs = slice(64 * h, 64 * (h + 1))
                    zri = tlp.tile([128, 512], F32, tag=f"zri{h}", name=f"zri_{c}_{h}")
                    nc.scalar.activation(
                        zri[64:128, :], pyh[h][64:128, :],
                        mybir.ActivationFunctionType.Ln,
                    )
                    nc.scalar.activation(
                        zri[64:128, :], zri[64:128, :],
                        mybir.ActivationFunctionType.Exp, scale=-1.0,
                    )
                    nc.vector.tensor_mul(yt[hs, :], pyh[h][0:64, :], zri[64:128, :])
                return yt

            def tail_wo(c, yt, drain=False):
                for mi in range(4):
                    ms = slice(128 * mi, 128 * (mi + 1))
                    r0 = 512 * c + 128 * mi
                    po = psp.tile([128, D], F32, tag="ps", name=f"po_{c}_{mi}")
                    for nn in range(2):
                        nc.tensor.matmul(
                            po[:, 512 * nn : 512 * (nn + 1)],
                            yt[:, ms],
                            wo_s[:, 512 * nn : 512 * (nn + 1)],
                            start=True,
                            stop=True,
                        )
                    ost = obp.tile([128, D], BF16, tag="ost", name=f"ost_{c}_{mi}")
                    # the drain chunk has no exp load left: split PSUM moves
                    # across ACT and DVE there; DVE-only elsewhere
                    if drain and mi % 2 == 0:
                        nc.scalar.copy(ost[:], po[:])
                    else:
                        nc.vector.tensor_copy(ost[:], po[:])
                    nc.sync.dma_start(out_d[r0 : r0 + 128, :], ost[:])

            # ---- chunk-pipelined emission. Chunk order 1,2,3,0: the drain
            # chunk is the 4-block one instead of the 16-block one (causality
            # permits chunk 0 any time after norm(0)). tail_norm(c) right
            # after the attention core, proj(next) between, tail_wo(c) after,
            # so PE never idles on the norm chain or the softmax-normalize.
            proj_norm_rot(0)
            emit_transposes()
            proj_norm_rot(1)
            yts = {}
            order = [1, 2, 3, 0]
            projq = [2, 3]
            for idx, c in enumerate(order):
                yts[c] = tail_norm(c, attention_core(c))
                if idx < len(projq):
                    proj_norm_rot(projq[idx])
                tail_wo(c, yts[c], drain=(idx == len(order) - 1))

    return nc


_NC = None
_NC_SPLIT = False


def _host_shards(x, Wq, Wk, Wv, Wo, s_qk):
    x = np.asarray(x, dtype=np.float32)
    Wq = np.asarray(Wq, dtype=np.float32)
    Wk = np.asarray(Wk, dtype=np.float32)
    Wv = np.asarray(Wv, dtype=np.float32)
    Wo = np.asarray(Wo, dtype=np.float32)
    s_qk = np.asarray(s_qk, dtype=np.float32)

    xT = np.ascontiguousarray(x.reshape(T, D).T).astype(NPBF16)

    dim_q = HD // 4
    freq = (1.0 / 1024.0) ** np.linspace(0.0, 1.0, dim_q, dtype=np.float32)
    freq = np.concatenate([freq, np.zeros(dim_q, np.float32)])
    theta = np.arange(T, dtype=np.float32)[:, None] * freq[None, :]
    cosT = np.cos(theta).T.astype(np.float32)
    sinT = np.sin(theta).T.astype(np.float32)
    A64 = np.concatenate([cosT, cosT], 0)
    B64 = np.concatenate([sinT, -sinT], 0)
    s_eff = s_qk * np.float32(np.sqrt(D))

    hselw = np.zeros((128, 128), np.float32)
    for h in range(HPC):
        hselw[64 * h : 64 * (h + 1), 64 * h : 64 * (h + 1)] = 1.0
    hselw = hselw.astype(NPBF16)

    in_maps = []
    for c in range(NCORES):
        cols = slice(EPC * c, EPC * (c + 1))
        rota_rows, rotb_rows = [], []
        for h in range(HPC):
            s = s_eff[HPC * c + h]
            s_swap = np.concatenate([s[32:], s[:32]])
            rota_rows.append(s[:, None] * A64)
            # pre-swap rows so the kernel's windowed muls read rotb at the
            # SOURCE rows (aligned with qn) while writing swapped outputs:
            # rotb'[b] = rotb_logical[swap(b)]
            rb = s_swap[:, None] * B64
            rotb_rows.append(np.concatenate([rb[32:], rb[:32]], 0))
        # double rotb per chunk: [p, chunk, {q,k}, t] with identical halves,
        # matching the kernel's combined q|k [128, 1024] norm stream
        rotb_sw = np.concatenate(rotb_rows, 0)
        rotb2 = np.empty((EPC, 2 * T), np.float32)
        for cc in range(NT):
            blk = rotb_sw[:, 512 * cc : 512 * (cc + 1)]
            rotb2[:, 1024 * cc : 1024 * cc + 512] = blk
            rotb2[:, 1024 * cc + 512 : 1024 * (cc + 1)] = blk
        in_maps.append(
            {
                "xT": xT,
                "wq": np.ascontiguousarray(Wq[:, cols]).astype(NPBF16),
                "wk": np.ascontiguousarray(Wk[:, cols]).astype(NPBF16),
                "wv": np.ascontiguousarray(Wv[:, cols]).astype(NPBF16),
                "wo": np.ascontiguousarray(Wo[EPC * c : EPC * (c + 1), :]).astype(NPBF16),
                "rota": np.concatenate(rota_rows, 0).astype(NPBF16),
                "rotb": rotb2.astype(NPBF16),
                "hselw": hselw,
            }
        )
    return in_maps


def _run_device(in_maps):
    global _NC, _NC_SPLIT
    _register_ntff_hook()
    if _NC is None:
        _NC = _build_nc()
    if not _NC_SPLIT:
        _split_ctrl_waits(_NC)
        _NC_SPLIT = True
    res = run_bass_kernel_spmd(_NC, in_maps, list(range(NCORES)))
    return (
        [np.asarray(r["out"]) for r in res.results],
        res.exec_time_ns,
        res.instructions_and_trace[1] if res.instructions_and_trace else None,
    )


def _worker(in_pkl, out_pkl):
    import pickle

    with open(in_pkl, "rb") as f:
        in_maps = pickle.load(f)
    outs, exec_ns, trace = _run_device(in_maps)
    with open(out_pkl, "wb") as f:
        pickle.dump({"outs": outs, "exec_time_ns": exec_ns, "trace": trace}, f)


def _run_subprocess(in_maps):
    import pickle
    import subprocess
    import tempfile

    d = tempfile.mkdtemp()
    in_pkl = os.path.join(d, "in.pkl")
    out_pkl = os.path.join(d, "out.pkl")
    with open(in_pkl, "wb") as f:
        pickle.dump(in_maps, f)
    here = os.path.dirname(os.path.abspath(__file__))
    code = (
        f"import sys; sys.path.insert(0, {here!r}); "
        f"import kernel; kernel._worker({in_pkl!r}, {out_pkl!r})"
    )
    subprocess.run([sys.executable, "-c", code], check=True, timeout=1800)
    with open(out_pkl, "rb") as f:
        out = pickle.load(f)
    return out["outs"], out["exec_time_ns"], out["trace"]


def _attempt(in_maps, use_subprocess):
    if use_subprocess:
        return _run_subprocess(in_maps)
    return _run_device(in_maps)


def kernel(x, Wq, Wk, Wv, Wo, s_qk):
    in_maps = _host_shards(x, Wq, Wk, Wv, Wo, s_qk)

    def total_of(outs):
        t = np.zeros((T, D), np.float64)
        for o in outs:
            t += o.astype(np.float64)
        return t

    # Run until two executions agree: device runs are deterministic, so a
    # mismatch flags the sporadic silent-corruption failure mode. Crashed
    # runs (NRT unrecoverable) poison this process's PJRT client, so later
    # attempts fall back to fresh subprocesses.
    results = []
    last_exc = None
    sub = False
    for attempt in range(5):
        try:
            outs, exec_ns, trace = _attempt(in_maps, sub)
        except Exception as e:
            last_exc = e
            sub = True
            continue
        t = total_of(outs)
        LAST["exec_time_ns"] = exec_ns
        LAST["trace"] = trace
        for tprev in results:
            denom = max(float(np.abs(tprev).max()), 1e-6)
            if float(np.abs(t - tprev).max()) <= 1e-4 * denom:
                return t.astype(np.float32).reshape(1, T, D)
        results.append(t)
    if results:
        return results[-1].astype(np.float32).reshape(1, T, D)
    raise last_exc
